# Initial kernel scaffold
#
"""FP8Linear on 8 Trainium2 NeuronCores (Bass/Tile, SPMD).

Reference math (per nn_FP8Linear):
    amax_x = max|x|, amax_w = max|w|               (global)
    x_scale = amax_x / C,  w_scale = max(amax_x, amax_w) / C,  C = fp32(448*0.8)
    out = dequant(e4m3fn(x/x_scale)) @ dequant(e4m3fn(w/w_scale)).T + bias

Sharding: tokens (B*S=8192) split across 8 cores (1024 each); weight
replicated. Inputs are staged pre-transposed ([Din, tok] / [Din, Dout]) so the
contraction dim lands on SBUF partitions with no on-device transposes.

Device per core (v2 -- front-phase compressed):
  - DMA priority: wsl (w amax slice) first, then x shard; the full-w stream is
    gated behind the last x chunk so the amax-feeding DMAs get the whole
    ~358GB/s. amax reduces are split DVE/GpSimd to shorten the serial tail.
  - local abs-max -> AllGather([1,2]->[8,2]) + 8-partition reduce (AG floor
    ~4.6us vs AllReduce ~9.7us)
  - scales: s = amax*(1/C); quantize t = x * rn(1/(2s)) -> TRN fp8e4.
    TRN fp8e4 max is +-240 (not e4m3fn's 448); quantizing x/(2s) instead of
    x/s keeps values <= 179.2 and exactly halves the e4m3fn grid, which the
    final output scale 4*s_x*s_w undoes.
  - x-quant on the (otherwise idle) Scalar/ACT engine, w-quant on DVE, so
    both streams feed the PE sooner.
  - fp8 matmuls accumulating over Din in PSUM; out = psum*(4 s_x s_w) + bias
"""
import numpy as np
from contextlib import ExitStack

import concourse.bacc as bacc
import concourse.bass as bass
import concourse.bass_isa as bass_isa
import concourse.tile as tile
from concourse import mybir
from concourse.bass_utils import run_bass_kernel_spmd
from concourse.tile_rust import add_dep_helper
import bass_rust

N_CORES = 8
B, S, DIN, DOUT = 4, 2048, 2048, 2048
TOK = B * S                  # 8192 tokens
TSH = TOK // N_CORES         # 1024 tokens per core
KT = DIN // 128              # 16 k-tiles
NT = DOUT // 512             # 4 n-groups
MT = TSH // 128              # 8 token-tiles

C_F64 = 448.0 * 0.8          # what jax sees before fp32 demotion
RC = float(np.float32(1.0 / np.float64(np.float32(C_F64))))  # rn(1/C)

USE_DOUBLE_ROW = True        # fp8 DoubleRow perf mode (2x PE, ~1e-4 extra err)
import os as _os
USE_AG = _os.environ.get("USE_AG", "1") == "1"       # AllGather vs AllReduce
XQ_SCALAR = _os.environ.get("XQ_SCALAR", "1") == "1"  # x-quant on ACT engine
GATE_W_DMA = _os.environ.get("GATE_W", "1") == "1"   # w stream waits for x
N_WARM = int(_os.environ.get("N_WARM", "0"))         # HAM warm-up matmuls
LDW_OPT = _os.environ.get("LDW_OPT", "0") == "1"     # walrus ldw dedupe pass
WT_BUFS = 4                  # fp32 w k-pair streaming slots (16KB/partition each)


def _patch_ldw_opt():
    """The walrus cmdline in bass_utils hardcodes --enable-ldw-opt=false;
    rewrite it so repeated-stationary matmuls skip the weight reload."""
    import concourse.bass_utils as bu
    if getattr(bu, "_ldw_opt_patched", False):
        return
    orig = bu.run_command

    def patched(cmd, *a, **kw):
        if isinstance(cmd, list):
            cmd = ["--enable-ldw-opt=true" if c == "--enable-ldw-opt=false"
                   else c for c in cmd]
        return orig(cmd, *a, **kw)

    bu.run_command = patched
    bu._ldw_opt_patched = True


if LDW_OPT:
    _patch_ldw_opt()

F32 = mybir.dt.float32
F8 = mybir.dt.float8e4

_built = None


def _build():
    global _built
    if _built is not None:
        return _built
    nc = bacc.Bacc("TRN2", target_bir_lowering=False, num_devices=N_CORES)

    xt_d = nc.dram_tensor("xt", [DIN, TSH], F32, kind="ExternalInput")
    wt_d = nc.dram_tensor("wt", [DIN, DOUT], F32, kind="ExternalInput")
    wsl_d = nc.dram_tensor("wsl", [128, DIN * DOUT // N_CORES // 128], F32,
                           kind="ExternalInput")   # [128, 4096]
    bias_d = nc.dram_tensor("bias", [1, DOUT], F32, kind="ExternalInput")
    out_d = nc.dram_tensor("out", [TSH, DOUT], F32, kind="ExternalOutput")

    WSLF = DIN * DOUT // N_CORES // 128  # 4096

    with tile.TileContext(nc) as tc, ExitStack() as ctx:
        pool = ctx.enter_context(tc.tile_pool(name="pool", bufs=1))
        opool = ctx.enter_context(tc.tile_pool(name="opool", bufs=4))
        psum = ctx.enter_context(tc.tile_pool(name="psum", bufs=8, space="PSUM"))
        dram = ctx.enter_context(tc.tile_pool(name="dram", bufs=1, space="DRAM"))
        wslctx = ExitStack()
        wslpool = wslctx.enter_context(tc.tile_pool(name="wslpool", bufs=1))

        # ---- input DMA: wsl (w amax slice) FIRST, then x shard in 4 chunks
        wsl_sb = wslpool.tile([128, WSLF], F32)
        with nc.named_scope("dma_wsl"):
            wsl_dma = nc.sync.dma_start(wsl_sb[:], wsl_d[:])
        xt_sb = pool.tile([128, KT, TSH], F32)
        xt_view = xt_d[:].rearrange("(k p) c -> p k c", p=128)
        x_dmas = []
        NXC = 8                                  # x DMA/amax chunks (2 k-tiles)
        with nc.named_scope("dma_x"):
            for c in range(NXC):
                x_dmas.append(nc.sync.dma_start(xt_sb[:, 2 * c:2 * c + 2, :],
                                                xt_view[:, 2 * c:2 * c + 2, :]))
        bias_bc = pool.tile([128, DOUT], F32)

        # ---- local abs-max (wsl first, then x chunks as their DMAs land)
        red = pool.tile([128, NXC + 1], F32)
        nc.vector.memset(red[:], 0.0)
        with nc.named_scope("amax"):
            nc.vector.tensor_reduce(red[:, NXC:NXC + 1], wsl_sb[:],
                                    mybir.AxisListType.X,
                                    mybir.AluOpType.max,
                                    apply_absolute_value=True)
            for c in range(NXC):
                nc.vector.tensor_reduce(red[:, c:c + 1],
                                        xt_sb[:, 2 * c:2 * c + 2, :],
                                        mybir.AxisListType.XY,
                                        mybir.AluOpType.max,
                                        apply_absolute_value=True)
            am2 = pool.tile([128, 2], F32)
            nc.vector.tensor_reduce(am2[:, 0:1], red[:, 0:NXC],
                                    mybir.AxisListType.X, mybir.AluOpType.max)
            nc.vector.tensor_copy(am2[:, 1:2], red[:, NXC:NXC + 1])
            # cross-partition: result lands on every partition
            amg_loc = pool.tile([128, 2], F32)
            nc.gpsimd.partition_all_reduce(amg_loc[:], am2[:], 128,
                                           bass_isa.ReduceOp.max)

        # ---- global amax across the 8 cores (8B payload)
        ag = pool.tile([128, 2], F32)
        with nc.named_scope("cc_amax"):
            if USE_AG:
                cc_in = dram.tile([1, 2], F32)
                cc_out = dram.tile([8, 2], F32)
                ag8 = pool.tile([8, 2], F32)
                nc.sync.dma_start(cc_in[:], amg_loc[0:1, :])
                nc.gpsimd.collective_compute(
                    "AllGather", mybir.AluOpType.bypass,
                    replica_groups=[list(range(N_CORES))],
                    ins=[cc_in[:].opt()], outs=[cc_out[:].opt()])
                ag_dma = nc.sync.dma_start(ag8[:], cc_out[:])
                agr = pool.tile([8, 2], F32)
                nc.gpsimd.partition_all_reduce(agr[:], ag8[:], 8,
                                               bass_isa.ReduceOp.max)
                nc.gpsimd.partition_broadcast(ag[:], agr[0:1, :])
            else:
                cc_in = dram.tile([1, 2], F32)
                cc_out = dram.tile([1, 2], F32)
                ag1 = pool.tile([1, 2], F32)
                nc.sync.dma_start(cc_in[:], amg_loc[0:1, :])
                nc.gpsimd.collective_compute(
                    "AllReduce", mybir.AluOpType.max,
                    replica_groups=[list(range(N_CORES))],
                    ins=[cc_in[:].opt()], outs=[cc_out[:].opt()])
                ag_dma = nc.sync.dma_start(ag1[:], cc_out[:])
                nc.gpsimd.partition_broadcast(ag[:], ag1[:])

        # ---- scales (every partition computes the same values)
        scal = pool.tile([128, 8], F32)  # [s_x, s_w, rx, rw, sc4, ...]
        with nc.named_scope("scales"):
            nc.vector.tensor_scalar(scal[:, 0:1], ag[:, 0:1], RC, None,
                                    mybir.AluOpType.mult)       # s_x
            mx = pool.tile([128, 1], F32)
            nc.vector.tensor_tensor(mx[:], ag[:, 0:1], ag[:, 1:2],
                                    mybir.AluOpType.max)
            nc.vector.tensor_scalar(scal[:, 1:2], mx[:], RC, None,
                                    mybir.AluOpType.mult)       # s_w
            d2x = pool.tile([128, 2], F32)
            nc.vector.tensor_scalar(d2x[:, 0:1], scal[:, 0:1], 2.0, None,
                                    mybir.AluOpType.mult)
            nc.vector.tensor_scalar(d2x[:, 1:2], scal[:, 1:2], 2.0, None,
                                    mybir.AluOpType.mult)
            nc.vector.reciprocal(scal[:, 2:4], d2x[:])          # rx, rw
            ss = pool.tile([128, 1], F32)
            nc.vector.tensor_tensor(ss[:], scal[:, 0:1], scal[:, 1:2],
                                    mybir.AluOpType.mult)
            nc.vector.tensor_scalar(scal[:, 4:5], ss[:], 4.0, None,
                                    mybir.AluOpType.mult)       # 4*s_x*s_w
        # ---- HAM warm-up: ~4us of junk matmuls gated on the collective
        # result, so the PE leaves its cold 1.2GHz state while quant runs
        warm_lhs = pool.tile([128, 8], F8)
        warm_rhs = pool.tile([128, 512], F8)
        nc.vector.memset(warm_rhs[:], 0.0)
        nc.vector.memset(warm_lhs[:], 0.0)
        nc.vector.tensor_copy(warm_lhs[:, 0:2], ag[:, 0:2])
        warm_ps = psum.tile([128, 512], F32, name="ps")
        for _ in range(N_WARM):
            nc.tensor.matmul(warm_ps[0:8, :], warm_lhs[:], warm_rhs[:],
                             start=True, stop=True)

        # ---- bias broadcast: stage into wsl_sb row 0 (dead after amax pass)
        nc.sync.dma_start(wsl_sb[0:1, 0:DOUT], bias_d[:])
        nc.gpsimd.partition_broadcast(bias_bc[:], wsl_sb[0:1, 0:DOUT])
        # release wsl's 16KB/partition so the w streaming pool can use it
        wslctx.close()
        wpool = ctx.enter_context(tc.tile_pool(name="wpool", bufs=WT_BUFS))

        # ---- stream w (transposed) k-tiles; quantize x (ACT) and w (DVE)
        xq = pool.tile([128, KT, TSH], F8)
        wq = pool.tile([128, KT, DOUT], F8)
        wt_view = wt_d[:].rearrange("(k p) c -> p k c", p=128)
        with nc.named_scope("quant"):
            for kk in range(KT // 2):
                wt_t = wpool.tile([128, 2, DOUT], F32, name="wt_t")
                wdma = nc.sync.dma_start(wt_t[:],
                                         wt_view[:, 2 * kk:2 * kk + 2, :])
                if GATE_W_DMA and kk < WT_BUFS:
                    add_dep_helper(wdma.ins, x_dmas[-1].ins, sync=True,
                                   reason="amax-feeding DMAs get priority")
                if XQ_SCALAR:
                    nc.scalar.activation(
                        xq[:, 2 * kk:2 * kk + 2, :],
                        xt_sb[:, 2 * kk:2 * kk + 2, :],
                        bass_rust.ActivationFunctionType.Copy,
                        bias=0.0, scale=scal[:, 2:3])
                else:
                    nc.vector.tensor_scalar(xq[:, 2 * kk:2 * kk + 2, :],
                                            xt_sb[:, 2 * kk:2 * kk + 2, :],
                                            scal[:, 2:3], None,
                                            mybir.AluOpType.mult)
                nc.vector.tensor_scalar(wq[:, 2 * kk:2 * kk + 2, :], wt_t[:],
                                        scal[:, 3:4], None,
                                        mybir.AluOpType.mult)

        # ---- matmuls + output scale/bias
        # groups of one token-tile m x 4 n-tiles = 4 PSUM banks; with
        # bufs=8 two groups are in flight so bank recycling (STT drain)
        # never stalls the PE. n is innermost: 4 consecutive matmuls share
        # the same stationary tile.
        with nc.named_scope("mm"):
            for m in range(MT):
                ptiles = [psum.tile([128, 512], F32, name="ps")
                          for _ in range(NT)]
                if USE_DOUBLE_ROW:
                    for kk in range(KT // 2):
                        for n in range(NT):
                            nc.tensor.matmul(
                                ptiles[n][:],
                                xq[:, 2 * kk:2 * kk + 2,
                                   m * 128:(m + 1) * 128],
                                wq[:, 2 * kk:2 * kk + 2,
                                   n * 512:(n + 1) * 512],
                                start=(kk == 0), stop=(kk == KT // 2 - 1),
                                perf_mode=mybir.MatmulPerfMode.DoubleRow)
                else:
                    for kk in range(KT):
                        for n in range(NT):
                            nc.tensor.matmul(
                                ptiles[n][:],
                                xq[:, kk, m * 128:(m + 1) * 128],
                                wq[:, kk, n * 512:(n + 1) * 512],
                                start=(kk == 0), stop=(kk == KT - 1))
                # last m-group: drain in 256-col pieces for a shorter tail
                PIECES = 2 if m == MT - 1 else 1
                W = 512 // PIECES
                for n in range(NT):
                    for p in range(PIECES):
                        osb = opool.tile([128, W], F32, name="osb")
                        c0 = n * 512 + p * W
                        nc.vector.scalar_tensor_tensor(
                            osb[:], ptiles[n][:, p * W:(p + 1) * W],
                            scal[:, 4:5], bias_bc[:, c0:c0 + W],
                            mybir.AluOpType.mult, mybir.AluOpType.add)
                        nc.sync.dma_start(
                            out_d[m * 128:(m + 1) * 128, c0:c0 + W], osb[:])

    nc.compile()
    _built = nc
    return nc


def kernel(x, weight, bias):
    x = np.asarray(x, dtype=np.float32)
    weight = np.asarray(weight, dtype=np.float32)
    bias = np.asarray(bias, dtype=np.float32)
    x2 = np.ascontiguousarray(x.reshape(TOK, DIN))
    wt = np.ascontiguousarray(weight.T)                    # [DIN, DOUT]
    rows = DOUT // N_CORES                                 # 256
    in_maps = []
    for i in range(N_CORES):
        in_maps.append({
            "xt": np.ascontiguousarray(x2[i * TSH:(i + 1) * TSH].T),
            "wt": wt,
            "wsl": np.ascontiguousarray(
                weight[i * rows:(i + 1) * rows]).reshape(128, -1),
            "bias": np.ascontiguousarray(bias.reshape(1, DOUT)),
        })
    nc = _build()
    br = run_bass_kernel_spmd(nc, in_maps, list(range(N_CORES)))
    out = np.concatenate([r["out"] for r in br.results], axis=0)
    return np.ascontiguousarray(out.reshape(B, S, DOUT))



# revision 1
# speedup vs baseline: 2.0668x; 2.0668x over previous
"""FP8Linear on 8 Trainium2 NeuronCores (Bass/Tile, SPMD).

Reference math (per nn_FP8Linear):
    amax_x = max|x|, amax_w = max|w|               (global)
    x_scale = amax_x / C,  w_scale = max(amax_x, amax_w) / C,  C = fp32(448*0.8)
    out = dequant(e4m3fn(x/x_scale)) @ dequant(e4m3fn(w/w_scale)).T + bias

Sharding: tokens (B*S=8192) split across 8 cores (1024 each); weight
replicated. Inputs are staged pre-transposed ([Din, tok] / [Din, Dout]) so the
contraction dim lands on SBUF partitions with no on-device transposes.

Device per core (v2 -- front-phase compressed):
  - DMA priority: wsl (w amax slice) first, then x shard; the full-w stream is
    gated behind the last x chunk so the amax-feeding DMAs get the whole
    ~358GB/s. amax reduces are split DVE/GpSimd to shorten the serial tail.
  - local abs-max -> AllGather([1,2]->[8,2]) + 8-partition reduce (AG floor
    ~4.6us vs AllReduce ~9.7us)
  - scales: s = amax*(1/C); quantize t = x * rn(1/(2s)) -> TRN fp8e4.
    TRN fp8e4 max is +-240 (not e4m3fn's 448); quantizing x/(2s) instead of
    x/s keeps values <= 179.2 and exactly halves the e4m3fn grid, which the
    final output scale 4*s_x*s_w undoes.
  - x-quant on the (otherwise idle) Scalar/ACT engine, w-quant on DVE, so
    both streams feed the PE sooner.
  - fp8 matmuls accumulating over Din in PSUM; out = psum*(4 s_x s_w) + bias
"""
import numpy as np
from contextlib import ExitStack

import concourse.bacc as bacc
import concourse.bass as bass
import concourse.bass_isa as bass_isa
import concourse.tile as tile
from concourse import mybir
from concourse.bass_utils import run_bass_kernel_spmd
from concourse.tile_rust import add_dep_helper
import bass_rust

N_CORES = 8
B, S, DIN, DOUT = 4, 2048, 2048, 2048
TOK = B * S                  # 8192 tokens
TSH = TOK // N_CORES         # 1024 tokens per core
KT = DIN // 128              # 16 k-tiles
NT = DOUT // 512             # 4 n-groups
MT = TSH // 128              # 8 token-tiles

C_F64 = 448.0 * 0.8          # what jax sees before fp32 demotion
RC = float(np.float32(1.0 / np.float64(np.float32(C_F64))))  # rn(1/C)

USE_DOUBLE_ROW = True        # fp8 DoubleRow perf mode (2x PE, ~1e-4 extra err)
import os as _os
USE_AG = _os.environ.get("USE_AG", "1") == "1"       # AllGather vs AllReduce
XQ_SCALAR = _os.environ.get("XQ_SCALAR", "1") == "1"  # x-quant on ACT engine
GATE_W_DMA = _os.environ.get("GATE_W", "1") == "1"   # w stream waits for x
N_WARM = int(_os.environ.get("N_WARM", "0"))         # HAM warm-up matmuls
LDW_OPT = _os.environ.get("LDW_OPT", "0") == "1"     # walrus ldw dedupe pass
WT_BUFS = 4                  # fp32 w k-pair streaming slots (16KB/partition each)


def _patch_ldw_opt():
    """The walrus cmdline in bass_utils hardcodes --enable-ldw-opt=false;
    rewrite it so repeated-stationary matmuls skip the weight reload."""
    import concourse.bass_utils as bu
    if getattr(bu, "_ldw_opt_patched", False):
        return
    orig = bu.run_command

    def patched(cmd, *a, **kw):
        if isinstance(cmd, list):
            cmd = ["--enable-ldw-opt=true" if c == "--enable-ldw-opt=false"
                   else c for c in cmd]
        return orig(cmd, *a, **kw)

    bu.run_command = patched
    bu._ldw_opt_patched = True


if LDW_OPT:
    _patch_ldw_opt()

F32 = mybir.dt.float32
F8 = mybir.dt.float8e4

_built = None


def _build():
    global _built
    if _built is not None:
        return _built
    nc = bacc.Bacc("TRN2", target_bir_lowering=False, num_devices=N_CORES)

    xt_d = nc.dram_tensor("xt", [DIN, TSH], F32, kind="ExternalInput")
    wt_d = nc.dram_tensor("wt", [DIN, DOUT], F32, kind="ExternalInput")
    wsl_d = nc.dram_tensor("wsl", [128, DIN * DOUT // N_CORES // 128], F32,
                           kind="ExternalInput")   # [128, 4096]
    bias_d = nc.dram_tensor("bias", [1, DOUT], F32, kind="ExternalInput")
    out_d = nc.dram_tensor("out", [TSH, DOUT], F32, kind="ExternalOutput")

    WSLF = DIN * DOUT // N_CORES // 128  # 4096

    with tile.TileContext(nc) as tc, ExitStack() as ctx:
        pool = ctx.enter_context(tc.tile_pool(name="pool", bufs=1))
        opool = ctx.enter_context(tc.tile_pool(name="opool", bufs=4))
        psum = ctx.enter_context(tc.tile_pool(name="psum", bufs=8, space="PSUM"))
        dram = ctx.enter_context(tc.tile_pool(name="dram", bufs=1, space="DRAM"))
        wslctx = ExitStack()
        wslpool = wslctx.enter_context(tc.tile_pool(name="wslpool", bufs=1))

        # ---- input DMA: wsl (w amax slice) FIRST, then x shard in 4 chunks
        wsl_sb = wslpool.tile([128, WSLF], F32)
        with nc.named_scope("dma_wsl"):
            wsl_dma = nc.sync.dma_start(wsl_sb[:], wsl_d[:])
        xt_sb = pool.tile([128, KT, TSH], F32)
        xt_view = xt_d[:].rearrange("(k p) c -> p k c", p=128)
        x_dmas = []
        NXC = 8                                  # x DMA/amax chunks (2 k-tiles)
        with nc.named_scope("dma_x"):
            for c in range(NXC):
                x_dmas.append(nc.sync.dma_start(xt_sb[:, 2 * c:2 * c + 2, :],
                                                xt_view[:, 2 * c:2 * c + 2, :]))
        bias_bc = pool.tile([128, DOUT], F32)

        # ---- local abs-max (wsl first, then x chunks as their DMAs land)
        red = pool.tile([128, NXC + 1], F32)
        nc.vector.memset(red[:], 0.0)
        with nc.named_scope("amax"):
            nc.vector.tensor_reduce(red[:, NXC:NXC + 1], wsl_sb[:],
                                    mybir.AxisListType.X,
                                    mybir.AluOpType.max,
                                    apply_absolute_value=True)
            for c in range(NXC):
                nc.vector.tensor_reduce(red[:, c:c + 1],
                                        xt_sb[:, 2 * c:2 * c + 2, :],
                                        mybir.AxisListType.XY,
                                        mybir.AluOpType.max,
                                        apply_absolute_value=True)
            am2 = pool.tile([128, 2], F32)
            nc.vector.tensor_reduce(am2[:, 0:1], red[:, 0:NXC],
                                    mybir.AxisListType.X, mybir.AluOpType.max)
            nc.vector.tensor_copy(am2[:, 1:2], red[:, NXC:NXC + 1])
            # cross-partition: result lands on every partition
            amg_loc = pool.tile([128, 2], F32)
            nc.gpsimd.partition_all_reduce(amg_loc[:], am2[:], 128,
                                           bass_isa.ReduceOp.max)

        # ---- global amax across the 8 cores (8B payload)
        ag = pool.tile([128, 2], F32)
        with nc.named_scope("cc_amax"):
            if USE_AG:
                cc_in = dram.tile([1, 2], F32)
                cc_out = dram.tile([8, 2], F32)
                ag8 = pool.tile([8, 2], F32)
                nc.sync.dma_start(cc_in[:], amg_loc[0:1, :])
                nc.gpsimd.collective_compute(
                    "AllGather", mybir.AluOpType.bypass,
                    replica_groups=[list(range(N_CORES))],
                    ins=[cc_in[:].opt()], outs=[cc_out[:].opt()])
                ag_dma = nc.sync.dma_start(ag8[:], cc_out[:])
                agr = pool.tile([8, 2], F32)
                nc.gpsimd.partition_all_reduce(agr[:], ag8[:], 8,
                                               bass_isa.ReduceOp.max)
                nc.gpsimd.partition_broadcast(ag[:], agr[0:1, :])
            else:
                cc_in = dram.tile([1, 2], F32)
                cc_out = dram.tile([1, 2], F32)
                ag1 = pool.tile([1, 2], F32)
                nc.sync.dma_start(cc_in[:], amg_loc[0:1, :])
                nc.gpsimd.collective_compute(
                    "AllReduce", mybir.AluOpType.max,
                    replica_groups=[list(range(N_CORES))],
                    ins=[cc_in[:].opt()], outs=[cc_out[:].opt()])
                ag_dma = nc.sync.dma_start(ag1[:], cc_out[:])
                nc.gpsimd.partition_broadcast(ag[:], ag1[:])

        # ---- scales (every partition computes the same values)
        scal = pool.tile([128, 8], F32)  # [s_x, s_w, rx, rw, sc4, ...]
        with nc.named_scope("scales"):
            nc.vector.tensor_scalar(scal[:, 0:1], ag[:, 0:1], RC, None,
                                    mybir.AluOpType.mult)       # s_x
            mx = pool.tile([128, 1], F32)
            nc.vector.tensor_tensor(mx[:], ag[:, 0:1], ag[:, 1:2],
                                    mybir.AluOpType.max)
            nc.vector.tensor_scalar(scal[:, 1:2], mx[:], RC, None,
                                    mybir.AluOpType.mult)       # s_w
            d2x = pool.tile([128, 2], F32)
            nc.vector.tensor_scalar(d2x[:, 0:1], scal[:, 0:1], 2.0, None,
                                    mybir.AluOpType.mult)
            nc.vector.tensor_scalar(d2x[:, 1:2], scal[:, 1:2], 2.0, None,
                                    mybir.AluOpType.mult)
            nc.vector.reciprocal(scal[:, 2:4], d2x[:])          # rx, rw
            ss = pool.tile([128, 1], F32)
            nc.vector.tensor_tensor(ss[:], scal[:, 0:1], scal[:, 1:2],
                                    mybir.AluOpType.mult)
            nc.vector.tensor_scalar(scal[:, 4:5], ss[:], 4.0, None,
                                    mybir.AluOpType.mult)       # 4*s_x*s_w
        # ---- HAM warm-up: ~4us of junk matmuls gated on the collective
        # result, so the PE leaves its cold 1.2GHz state while quant runs
        warm_lhs = pool.tile([128, 8], F8)
        warm_rhs = pool.tile([128, 512], F8)
        nc.vector.memset(warm_rhs[:], 0.0)
        nc.vector.memset(warm_lhs[:], 0.0)
        nc.vector.tensor_copy(warm_lhs[:, 0:2], ag[:, 0:2])
        warm_ps = psum.tile([128, 512], F32, name="ps")
        for _ in range(N_WARM):
            nc.tensor.matmul(warm_ps[0:8, :], warm_lhs[:], warm_rhs[:],
                             start=True, stop=True)

        # ---- bias broadcast: stage into wsl_sb row 0 (dead after amax pass)
        nc.sync.dma_start(wsl_sb[0:1, 0:DOUT], bias_d[:])
        nc.gpsimd.partition_broadcast(bias_bc[:], wsl_sb[0:1, 0:DOUT])
        # release wsl's 16KB/partition so the w streaming pool can use it
        wslctx.close()
        wpool = ctx.enter_context(tc.tile_pool(name="wpool", bufs=WT_BUFS))

        # ---- stream w (transposed) k-tiles; quantize x (ACT) and w (DVE)
        xq = pool.tile([128, KT, TSH], F8)
        wq = pool.tile([128, KT, DOUT], F8)
        wt_view = wt_d[:].rearrange("(k p) c -> p k c", p=128)
        with nc.named_scope("quant"):
            for kk in range(KT // 2):
                wt_t = wpool.tile([128, 2, DOUT], F32, name="wt_t")
                wdma = nc.sync.dma_start(wt_t[:],
                                         wt_view[:, 2 * kk:2 * kk + 2, :])
                if GATE_W_DMA and kk < WT_BUFS:
                    add_dep_helper(wdma.ins, x_dmas[-1].ins, sync=True,
                                   reason="amax-feeding DMAs get priority")
                if XQ_SCALAR:
                    nc.scalar.activation(
                        xq[:, 2 * kk:2 * kk + 2, :],
                        xt_sb[:, 2 * kk:2 * kk + 2, :],
                        bass_rust.ActivationFunctionType.Copy,
                        bias=0.0, scale=scal[:, 2:3])
                else:
                    nc.vector.tensor_scalar(xq[:, 2 * kk:2 * kk + 2, :],
                                            xt_sb[:, 2 * kk:2 * kk + 2, :],
                                            scal[:, 2:3], None,
                                            mybir.AluOpType.mult)
                nc.vector.tensor_scalar(wq[:, 2 * kk:2 * kk + 2, :], wt_t[:],
                                        scal[:, 3:4], None,
                                        mybir.AluOpType.mult)

        # ---- matmuls + output scale/bias
        # groups of one token-tile m x 4 n-tiles = 4 PSUM banks; with
        # bufs=8 two groups are in flight so bank recycling (STT drain)
        # never stalls the PE. n is innermost: 4 consecutive matmuls share
        # the same stationary tile.
        with nc.named_scope("mm"):
            for m in range(MT):
                ptiles = [psum.tile([128, 512], F32, name="ps")
                          for _ in range(NT)]
                if USE_DOUBLE_ROW:
                    for kk in range(KT // 2):
                        for n in range(NT):
                            nc.tensor.matmul(
                                ptiles[n][:],
                                xq[:, 2 * kk:2 * kk + 2,
                                   m * 128:(m + 1) * 128],
                                wq[:, 2 * kk:2 * kk + 2,
                                   n * 512:(n + 1) * 512],
                                start=(kk == 0), stop=(kk == KT // 2 - 1),
                                perf_mode=mybir.MatmulPerfMode.DoubleRow)
                else:
                    for kk in range(KT):
                        for n in range(NT):
                            nc.tensor.matmul(
                                ptiles[n][:],
                                xq[:, kk, m * 128:(m + 1) * 128],
                                wq[:, kk, n * 512:(n + 1) * 512],
                                start=(kk == 0), stop=(kk == KT - 1))
                # last m-group: drain in 256-col pieces for a shorter tail
                PIECES = 2 if m == MT - 1 else 1
                W = 512 // PIECES
                for n in range(NT):
                    for p in range(PIECES):
                        osb = opool.tile([128, W], F32, name="osb")
                        c0 = n * 512 + p * W
                        nc.vector.scalar_tensor_tensor(
                            osb[:], ptiles[n][:, p * W:(p + 1) * W],
                            scal[:, 4:5], bias_bc[:, c0:c0 + W],
                            mybir.AluOpType.mult, mybir.AluOpType.add)
                        nc.sync.dma_start(
                            out_d[m * 128:(m + 1) * 128, c0:c0 + W], osb[:])

    nc.compile()
    _built = nc
    return nc


def kernel(x, weight, bias):
    x = np.asarray(x, dtype=np.float32)
    weight = np.asarray(weight, dtype=np.float32)
    bias = np.asarray(bias, dtype=np.float32)
    x2 = np.ascontiguousarray(x.reshape(TOK, DIN))
    wt = np.ascontiguousarray(weight.T)                    # [DIN, DOUT]
    rows = DOUT // N_CORES                                 # 256
    in_maps = []
    for i in range(N_CORES):
        in_maps.append({
            "xt": np.ascontiguousarray(x2[i * TSH:(i + 1) * TSH].T),
            "wt": wt,
            "wsl": np.ascontiguousarray(
                weight[i * rows:(i + 1) * rows]).reshape(128, -1),
            "bias": np.ascontiguousarray(bias.reshape(1, DOUT)),
        })
    nc = _build()
    br = run_bass_kernel_spmd(nc, in_maps, list(range(N_CORES)))
    out = np.concatenate([r["out"] for r in br.results], axis=0)
    return np.ascontiguousarray(out.reshape(B, S, DOUT))

